# revision 24
# baseline (speedup 1.0000x reference)
"""Trainium2 Bass kernel for a 4-layer causal-attention LM.

Model: V=32000, D=1024, H=16 heads, L=4 layers, B=2, S=1024.
  x = emb[tokens] + pos_enc
  per layer: q,k,v = x@W; causal softmax attention; out-proj; residual; LN
  logits = x @ out_w

Sharding over 8 NeuronCores: DP=2 over batch x Megatron TP=4 over heads.
Core c: batch g=c//4, rank r=c%4 owns heads [4r, 4r+4) and vocab cols
[8000r, 8000(r+1)). QKV column-parallel; ctx AllGather per seq-chunk;
out-proj replicated on the gathered ctx (no AllReduce). Final vocab
projection column-parallel; host concatenates shards.

Perf structure (v4.7):
  - activations bf16 feature-major only (xTb)
  - causal-trimmed scores/exp/AV (diagonal tiles compute cols >= 128*trel)
  - score pairs row-group concurrent (K=64 at row groups 0/64)
  - st0 (feature-sum of attn_out) computed from LOCAL ctx pre-AllGather
    (owsum rank-slice, 2 matmuls) and shipped as a 257th row of the AG
    payload -> no post-AG matmul chain head-blocks the PE queue
  - DMA queue split: AG-gated loads (ctxF/st0row) on SP; weight/const
    prefetch on ACT queue; ag_in + collectives + gathers + broadcasts on
    Pool; all weight tensors host-relaid per-partition-contiguous so each
    DMA is 128 descriptors instead of 1024
  - sequential emission pipeline: every AllGather covered by 19-23us of
    ready PE work; DVE fences tie post-AG streams after attention's last
    write so the scheduler cannot head-block the DVE FIFO on a collective
  - LayerNorm entirely on DVE: mean via the AG-shipped st0, variance via
    one bf16 matmul chain, rstd via bit-trick + 2 Newton steps -> the only
    ACT table ever loaded is Exp (no mid-kernel table switches); -mean is
    broadcast and applied during the out-proj eviction phase
  - softmax normalization batched per head-pair (one reciprocal, one
    partition_broadcast per pair); q/k/final evictions on DVE to keep ACT
    exclusively for attention exp
  - final vocab projection weight tiles prefetch through a 6-slot ring of
    dead layer-phase tags
  - bf16 logits, host casts to f32
"""

import numpy as np

V, D, H, L = 32000, 1024, 16, 4
B, S = 2, 1024
HD = D // H            # 64
P = 128
NG = 4                 # TP degree (cores per batch group)
HL = H // NG           # 4 heads per core
HCOLS = HL * HD        # 256 projection cols per core
VS = V // NG           # 8000 vocab shard
DT = D // P            # 8 d-tiles
SQC = 512              # seq chunk
NSQC = S // SQC        # 2
NT = S // P            # 8 seq tiles
VC = 500               # vocab tile (8000 = 16*500)
NVC = VS // VC         # 16
AGR = HCOLS + 1        # AG payload rows: 256 ctx + 1 st0 partial
SCALE = 1.0 / float(np.sqrt(HD))
EPS = 1e-5
NEG = -1.0e9
RG = [[0, 1, 2, 3], [4, 5, 6, 7]]

_COMPILED = None


def _pos_encoding():
    pos = np.arange(S, dtype=np.float32)[:, None]
    div = np.exp(np.arange(0, D, 2, dtype=np.float32) * (-np.log(10000.0) / D))
    ang = pos * div
    pe = np.stack([np.sin(ang), np.cos(ang)], axis=-1).reshape(S, D)
    return pe.astype(np.float32)


def _build():
    import concourse.bass as bass
    import concourse.tile as tile
    from concourse import bacc, mybir

    f32 = mybir.dt.float32
    u32 = mybir.dt.uint32
    bf16 = mybir.dt.bfloat16
    i32 = mybir.dt.int32
    AF = mybir.ActivationFunctionType

    nc = bacc.Bacc("TRN2", target_bir_lowering=False, debug=False, num_devices=8)

    tok = nc.dram_tensor("tok", [S, 1], i32, kind="ExternalInput").ap()
    identb_d = nc.dram_tensor("identb", [P, P], bf16, kind="ExternalInput").ap()
    onesb_d = nc.dram_tensor("onesb", [P, 1], bf16, kind="ExternalInput").ap()
    maskd_d = nc.dram_tensor("maskd", [P, P], f32, kind="ExternalInput").ap()
    emb = nc.dram_tensor("emb", [V, D], bf16, kind="ExternalInput").ap()
    peT = nc.dram_tensor("peT", [D, S], bf16, kind="ExternalInput").ap()
    qw = nc.dram_tensor("qw", [L, D, HCOLS], bf16, kind="ExternalInput").ap()
    kw = nc.dram_tensor("kw", [L, D, HCOLS], bf16, kind="ExternalInput").ap()
    vw = nc.dram_tensor("vw", [L, D, HCOLS], bf16, kind="ExternalInput").ap()
    ow = nc.dram_tensor("ow", [L, D, D], bf16, kind="ExternalInput").ap()
    owsumr = nc.dram_tensor("owsumr", [L, HCOLS], bf16, kind="ExternalInput").ap()
    outw = nc.dram_tensor("outw", [D, VS], bf16, kind="ExternalInput").ap()
    out = nc.dram_tensor("out", [S, VS], bf16, kind="ExternalOutput").ap()

    with tile.TileContext(nc) as tc:
        with (
            tc.tile_pool(name="const", bufs=1) as constp,
            tc.tile_pool(name="xp", bufs=1) as xp,
            tc.tile_pool(name="psum", bufs=2, space="PSUM") as psp,
        ):
            ident = constp.tile([P, P], bf16)
            nc.scalar.dma_start(out=ident[:], in_=identb_d[:])
            onesb = constp.tile([P, 1], bf16)
            nc.scalar.dma_start(out=onesb[:], in_=onesb_d[:])
            epsb = constp.tile([1, 1], f32)
            nc.vector.memset(epsb[:], EPS)
            # additive causal mask for the 128-wide diagonal block:
            # maskd[i, j] = 0 if j >= i else NEG (same for every trel)
            maskd = constp.tile([P, P], f32)
            nc.scalar.dma_start(out=maskd[:], in_=maskd_d[:])

            # persistent activations, bf16 feature-major: x[d, s]
            xTb0 = xp.tile([P, DT, SQC], bf16, name="xTb0")
            xTb1 = xp.tile([P, DT, SQC], bf16, name="xTb1")
            xTbs = (xTb0, xTb1)
            sx0 = xp.tile([1, S], bf16, name="sx0")  # feature-sum of x0

            with (
                tc.tile_pool(name="wp", bufs=2) as wp,
                tc.tile_pool(name="owp", bufs=1) as owp,
                tc.tile_pool(name="apl", bufs=1) as apool,
                tc.tile_pool(name="expp", bufs=6) as expp,
                tc.tile_pool(name="lnp", bufs=1) as lnp,
                tc.tile_pool(name="dcp", bufs=2) as dcp,
                tc.tile_pool(name="small", bufs=1) as smallp,
                tc.tile_pool(name="dram", bufs=2, space="DRAM") as dramp,
            ):
                # ---- small collective warm-up: sync cores + init CC stack ----
                wi = dramp.tile([8, 8], bf16, tag="wagi", bufs=1, name="warm_in")
                wseed = constp.tile([8, 8], bf16)
                nc.gpsimd.memset(wseed[:], 0.0)
                nc.gpsimd.dma_start(out=wi[:], in_=wseed[:])
                wo = dramp.tile([NG * 8, 8], bf16, tag="wago", bufs=1, name="warm_out")
                nc.gpsimd.collective_compute(
                    "AllGather",
                    mybir.AluOpType.bypass,
                    replica_groups=RG,
                    ins=[wi[:].opt()],
                    outs=[wo[:].opt()],
                )

                def load_weights_qkv(l):
                    w = {}
                    for nm, src in (("qw", qw), ("kw", kw), ("vw", vw)):
                        t = wp.tile([P, DT, HCOLS], bf16, tag=nm, name=f"{nm}{l}")
                        nc.scalar.dma_start(
                            out=t[:], in_=src[l].rearrange("(a p) m -> p a m", p=P)
                        )
                        w[nm] = t
                    t = wp.tile([P, 2, 1], bf16, tag="owsr", name=f"owsr{l}")
                    nc.scalar.dma_start(
                        out=t[:],
                        in_=owsumr[l].rearrange("(hp p o) -> p hp o", p=P, o=1),
                    )
                    w["owsr"] = t
                    return w

                def load_weights_ow(l, w):
                    t = owp.tile([P, DT, D], bf16, tag="ow", name=f"ow{l}")
                    nc.scalar.dma_start(
                        out=t[:], in_=ow[l].rearrange("(a p) m -> p a m", p=P)
                    )
                    w["ow"] = t

                # ---- embedding: gather rows, transpose, +pe -> xTb ----
                w = load_weights_qkv(0)
                load_weights_ow(0, w)

                tokt = constp.tile([P, NT], i32)
                nc.sync.dma_start(
                    out=tokt[:], in_=tok.rearrange("(t p) o -> p (t o)", p=P)
                )

                def emb_gen(c, embp):
                    for st in range(4 * c, 4 * c + 4):
                        xrow = embp.tile([P, D], bf16, tag="xrow")
                        nc.gpsimd.indirect_dma_start(
                            out=xrow[:],
                            out_offset=None,
                            in_=emb[:],
                            in_offset=bass.IndirectOffsetOnAxis(
                                ap=tokt[:, st : st + 1], axis=0
                            ),
                        )
                        pesb = embp.tile([P, DT, P], bf16, tag="pesb")
                        nc.scalar.dma_start(
                            out=pesb[:],
                            in_=peT[:, st * P : (st + 1) * P].rearrange(
                                "(a p) s -> p a s", p=P
                            ),
                        )
                        half = xTbs[c]
                        lst = st % (NT // 2)
                        for dc in range(DT):
                            tps = psp.tile([P, P], bf16, tag="mm", name=f"t{st}_{dc}")
                            nc.tensor.transpose(
                                tps[:], xrow[:, dc * P : (dc + 1) * P], ident[:]
                            )
                            nc.vector.tensor_add(
                                half[:, dc, lst * P : (lst + 1) * P],
                                tps[:],
                                pesb[:, dc, :],
                            )
                            if dc % 2 == 1:
                                yield
                    # feature-sums of this chunk for the layer-0 LN mean
                    stp = psp.tile([1, SQC], f32, tag="mm", name=f"sx{c}")
                    for dc in range(DT):
                        nc.tensor.matmul(
                            stp[:],
                            lhsT=onesb[:],
                            rhs=xTbs[c][:, dc, :],
                            start=(dc == 0),
                            stop=(dc == DT - 1),
                        )
                    nc.scalar.copy(sx0[:, c * SQC : (c + 1) * SQC], stp[:])
                    yield

                def qkv_gen(l, c, w, stl):
                    # q,k feature-major [headcol, s]; v seq-major bf16 with a
                    # ones column at 64 for free softmax sums.
                    if c == 0:
                        stl["qT"] = apool.tile([P, 2, S], bf16, tag="qT", name=f"qT{l}")
                        stl["kT"] = apool.tile([P, 2, S], bf16, tag="kT", name=f"kT{l}")
                        stl["vS"] = apool.tile(
                            [P, NT, HL, 66], bf16, tag="vS", name=f"vS{l}"
                        )
                        stl["ctx"] = apool.tile(
                            [P, 2, S], bf16, tag="ctx", name=f"ctx{l}"
                        )
                    qT, kT, vS = stl["qT"], stl["kT"], stl["vS"]
                    xTb = xTbs[c]
                    for dst, wsb in ((qT, w["qw"]), (kT, w["kw"])):
                        for hp in range(2):
                            ps = psp.tile([P, SQC], f32, tag="mm")
                            for kt in range(DT):
                                nc.tensor.matmul(
                                    ps[:],
                                    lhsT=wsb[:, kt, hp * P : (hp + 1) * P],
                                    rhs=xTb[:, kt, :],
                                    start=(kt == 0),
                                    stop=(kt == DT - 1),
                                )
                            nc.vector.tensor_copy(
                                dst[:, hp, c * SQC : (c + 1) * SQC], ps[:]
                            )
                            yield
                    for st in range(4 * c, 4 * c + 4):
                        lt = st - 4 * c
                        nc.vector.memset(vS[:, st, :, 64:65], 1.0)
                        ps = psp.tile([P, HCOLS], f32, tag="mm")
                        for kt in range(DT):
                            nc.tensor.matmul(
                                ps[:],
                                lhsT=xTb[:, kt, lt * P : (lt + 1) * P],
                                rhs=w["vw"][:, kt, :],
                                start=(kt == 0),
                                stop=(kt == DT - 1),
                            )
                        nc.vector.tensor_copy(
                            vS[:, st, :, 0:64],
                            ps[:].rearrange("p (h e) -> p h e", h=HL),
                        )
                        yield

                def att_gen(l, c, stl):
                    # transposed scores [sk, sq], causal-trimmed; score pairs
                    # run concurrently in row groups 0/64; ctx feature-major
                    qT, kT, vS, ctx = stl["qT"], stl["kT"], stl["vS"], stl["ctx"]
                    nt_vis = 4 * c + 4
                    for hpair in range(HL // 2):
                        hs = (2 * hpair, 2 * hpair + 1)
                        avs = {}
                        for h in hs:
                            avs[h] = psp.tile(
                                [P, SQC], f32, tag="av", bufs=2, name=f"av{l}{c}{h}"
                            )
                        for t in range(nt_vis):
                            trel = t - 4 * c
                            lo = max(0, trel) * P
                            exs = {}
                            for h in hs:
                                hp, hr = divmod(h, 2)
                                p0 = 64 * hr
                                sc = psp.tile([P, SQC], f32, tag="sc", bufs=3)
                                nc.tensor.matmul(
                                    sc[:, lo:],
                                    lhsT=kT[p0 : p0 + 64, hp, t * P : (t + 1) * P],
                                    rhs=qT[
                                        p0 : p0 + 64, hp, c * SQC + lo : (c + 1) * SQC
                                    ],
                                    start=True,
                                    stop=True,
                                )
                                exs[h] = sc
                            yield
                            for h in hs:
                                sc = exs[h]
                                if trel >= 0:
                                    nc.vector.tensor_add(
                                        sc[:, lo : lo + P], sc[:, lo : lo + P], maskd[:]
                                    )
                                ex = expp.tile([P, SQC], bf16, tag="ex", bufs=6)
                                nc.scalar.activation(
                                    ex[:, lo:], sc[:, lo:], AF.Exp, scale=SCALE
                                )
                                exs[h] = ex
                            for h in hs:
                                nc.tensor.matmul(
                                    avs[h][0:65, lo:],
                                    lhsT=vS[:, t, h, 0:65],
                                    rhs=exs[h][:, lo:],
                                    start=(t == 0),
                                    stop=(t == nt_vis - 1),
                                )
                            yield
                        # batched normalization for the pair: one reciprocal,
                        # one partition_broadcast over [64, 2*SQC]
                        h0, h1 = hs
                        hp = h0 // 2
                        ssum2 = smallp.tile([1, 2 * SQC], f32, tag="ssum", bufs=2)
                        nc.vector.tensor_copy(ssum2[:, 0:SQC], avs[h0][64:65, :])
                        nc.vector.tensor_copy(ssum2[:, SQC:], avs[h1][64:65, :])
                        inv2 = smallp.tile([1, 2 * SQC], f32, tag="inv", bufs=2)
                        nc.vector.reciprocal_approx_fast(inv2[:], ssum2[:])
                        invb2 = smallp.tile([64, 2 * SQC], f32, tag="invb", bufs=2)
                        nc.gpsimd.partition_broadcast(invb2[:], inv2[:])
                        nc.vector.tensor_mul(
                            ctx[0:64, hp, c * SQC : (c + 1) * SQC],
                            avs[h0][0:64, :],
                            invb2[:, 0:SQC],
                        )
                        yield
                        nc.vector.tensor_mul(
                            ctx[64:128, hp, c * SQC : (c + 1) * SQC],
                            avs[h1][0:64, :],
                            invb2[:, SQC:],
                        )
                        yield

                def ag_ctx(l, c, w, stl):
                    ctx = stl["ctx"]
                    # half A: head-pair 0 only -> op chains can start on the
                    # even kt tiles while half B still flies
                    ag_inA = dramp.tile(
                        [P, SQC], bf16, tag="agiA", name=f"agiA{l}_{c}"
                    )
                    nc.gpsimd.dma_start(
                        out=ag_inA[:], in_=ctx[:, 0, c * SQC : (c + 1) * SQC]
                    )
                    ag_outA = dramp.tile(
                        [NG * P, SQC], bf16, tag="agoA", name=f"agoA{l}_{c}"
                    )
                    nc.gpsimd.collective_compute(
                        "AllGather",
                        mybir.AluOpType.bypass,
                        replica_groups=RG,
                        ins=[ag_inA[:].opt()],
                        outs=[ag_outA[:].opt()],
                    )
                    # st0 partial = owsum_rank . ctx_local  (2 matmuls)
                    st0p = psp.tile([1, SQC], f32, tag="mm", name=f"st0p{l}{c}")
                    for hp in range(2):
                        nc.tensor.matmul(
                            st0p[:],
                            lhsT=w["owsr"][:, hp, :],
                            rhs=ctx[:, hp, c * SQC : (c + 1) * SQC],
                            start=(hp == 0),
                            stop=(hp == 1),
                        )
                    st0l = smallp.tile([1, SQC], bf16, tag="st0l", bufs=2)
                    nc.scalar.copy(st0l[:], st0p[:])
                    # half B: head-pair 1 + the st0 row
                    ag_inB = dramp.tile(
                        [P + 1, SQC], bf16, tag="agiB", name=f"agiB{l}_{c}"
                    )
                    nc.gpsimd.dma_start(
                        out=ag_inB[0:P, :], in_=ctx[:, 1, c * SQC : (c + 1) * SQC]
                    )
                    nc.gpsimd.dma_start(out=ag_inB[P : P + 1, :], in_=st0l[:])
                    ag_outB = dramp.tile(
                        [NG * (P + 1), SQC], bf16, tag="agoB", name=f"agoB{l}_{c}"
                    )
                    nc.gpsimd.collective_compute(
                        "AllGather",
                        mybir.AluOpType.bypass,
                        replica_groups=RG,
                        ins=[ag_inB[:].opt()],
                        outs=[ag_outB[:].opt()],
                    )
                    stl[f"ag{c}"] = (ag_outA, ag_outB)

                def oprln_gen(l, c, w, stl, fence=None):
                    # replicated out-proj on gathered ctx; residual fused into
                    # psum eviction; st0 arrives with the AG payload; st1 via
                    # one bf16 matmul chain; writes xTb[c]
                    ag_outA, ag_outB = stl[f"ag{c}"]
                    agovB = ag_outB[:].rearrange("(g x) s -> g x s", g=NG)
                    # st0 partial rows [NG, SQC] -> one partition
                    st0g = smallp.tile([1, NG, SQC], bf16, tag=f"st0g{c}")
                    if fence is not None:
                        # scheduling fence: ties this chunk's post-AG DVE
                        # stream strictly after the predecessor stream's last
                        # DVE write, so the scheduler cannot head-block the
                        # DVE FIFO on the collective
                        nc.vector.tensor_copy(st0g[0:1, 0:1, 0:1], fence)
                    nc.sync.dma_start(
                        out=st0g[:],
                        in_=agovB[:, P : P + 1, :].rearrange("g o s -> o g s"),
                    )
                    # gathered ctx, feature-major; half A fills even kt
                    # tiles, half B the odd ones
                    ctxF = lnp.tile([P, DT, SQC], bf16, tag="ctxF", bufs=2)
                    ctxv = ctxF[:].rearrange("p (g a) s -> p a g s", a=2)
                    nc.sync.dma_start(
                        out=ctxv[:, 0, :, :],
                        in_=ag_outA[:].rearrange("(g p) s -> p g s", p=P),
                    )
                    nc.sync.dma_start(
                        out=ctxv[:, 1, :, :],
                        in_=agovB[:, 0:P, :].rearrange("g p s -> p g s"),
                    )
                    # st0 = sum of the four partials (+ sx0 for layer 0)
                    st0f = smallp.tile([1, SQC], f32, tag=f"st0f{c}")
                    nc.vector.tensor_add(st0f[:], st0g[:, 0, :], st0g[:, 1, :])
                    nc.vector.tensor_add(st0f[:], st0f[:], st0g[:, 2, :])
                    nc.vector.tensor_add(st0f[:], st0f[:], st0g[:, 3, :])
                    if l == 0:
                        nc.vector.tensor_add(
                            st0f[:], st0f[:], sx0[:, c * SQC : (c + 1) * SQC]
                        )
                    # -mean: broadcast EARLY (ready before op chains end)
                    nmean = smallp.tile([1, SQC], bf16, tag=f"nmean{c}")
                    nc.vector.tensor_scalar_mul(nmean[:], st0f[:], -1.0 / D)
                    mb = smallp.tile([P, SQC], bf16, tag=f"mb{c}")
                    nc.gpsimd.partition_broadcast(mb[:], nmean[:])
                    # (st0/D)^2 == (-mean)^2, no Square activation needed
                    msqd = smallp.tile([1, SQC], f32, tag=f"msqd{c}")
                    nc.vector.tensor_mul(msqd[:], nmean[:], nmean[:])
                    yield
                    xr = lnp.tile([P, DT, SQC], bf16, tag=f"xr{c}")
                    sqtb = lnp.tile([P, DT, SQC], bf16, tag=f"sq{c}")
                    for dc in range(DT):
                        ps = psp.tile([P, SQC], f32, tag="mm", bufs=2)
                        kts = list(range(0, DT, 2)) + list(range(1, DT, 2))
                        for ki, kt in enumerate(kts):
                            nc.tensor.matmul(
                                ps[:],
                                lhsT=w["ow"][:, kt, dc * P : (dc + 1) * P],
                                rhs=ctxF[:, kt, :],
                                start=(ki == 0),
                                stop=(ki == DT - 1),
                            )
                        nc.vector.tensor_add(xr[:, dc, :], ps[:], xTbs[c][:, dc, :])
                        nc.vector.tensor_mul(sqtb[:, dc, :], xr[:, dc, :], xr[:, dc, :])
                        # pre-add the mean while the tail stats are in flight
                        nc.vector.tensor_add(xr[:, dc, :], xr[:, dc, :], mb[:])
                        yield
                    st1 = psp.tile([1, SQC], f32, tag="st", bufs=1, name=f"st_{l}{c}")
                    for kt in range(DT):
                        nc.tensor.matmul(
                            st1[:],
                            lhsT=onesb[:],
                            rhs=sqtb[:, kt, :],
                            start=(kt == 0),
                            stop=(kt == DT - 1),
                        )
                    yield
                    # vpe = st1/D + eps  (DVE, reads PSUM; no ACT tables)
                    vpe = smallp.tile([1, SQC], f32, tag=f"ex2{c}")
                    nc.vector.tensor_scalar(
                        vpe[:], st1[:], 1.0 / D, EPS,
                        op0=mybir.AluOpType.mult, op1=mybir.AluOpType.add,
                    )
                    nc.vector.tensor_sub(vpe[:], vpe[:], msqd[:])
                    # rstd = rsqrt(vpe) via bit-trick seed + 2 Newton steps
                    yi = smallp.tile([1, SQC], u32, tag="qki")
                    nc.vector.tensor_scalar(
                        yi[:], vpe[:].bitcast(u32), 1, None,
                        op0=mybir.AluOpType.logical_shift_right,
                    )
                    af = smallp.tile([1, SQC], f32, tag="qkaf")
                    nc.vector.tensor_copy(af[:], yi[:])
                    nc.vector.tensor_scalar(
                        af[:], af[:], -1.0, float(0x5F3759DF),
                        op0=mybir.AluOpType.mult, op1=mybir.AluOpType.add,
                    )
                    nc.vector.tensor_copy(yi[:], af[:])
                    y0 = yi[:].bitcast(f32)
                    t1 = smallp.tile([1, SQC], f32, tag="qkt")
                    y1 = smallp.tile([1, SQC], f32, tag="qky")
                    nc.vector.tensor_mul(t1[:], y0, y0)
                    nc.vector.tensor_mul(t1[:], t1[:], vpe[:])
                    nc.vector.tensor_scalar(
                        t1[:], t1[:], -0.5, 1.5,
                        op0=mybir.AluOpType.mult, op1=mybir.AluOpType.add,
                    )
                    nc.vector.tensor_mul(y1[:], y0, t1[:])
                    nc.vector.tensor_mul(t1[:], y1[:], y1[:])
                    nc.vector.tensor_mul(t1[:], t1[:], vpe[:])
                    nc.vector.tensor_scalar(
                        t1[:], t1[:], -0.5, 1.5,
                        op0=mybir.AluOpType.mult, op1=mybir.AluOpType.add,
                    )
                    rstd = smallp.tile([1, SQC], bf16, tag=f"rstd{c}")
                    nc.vector.tensor_mul(rstd[:], y1[:], t1[:])
                    rb = smallp.tile([P, SQC], bf16, tag=f"rb{c}")
                    nc.gpsimd.partition_broadcast(rb[:], rstd[:])
                    for dc in range(DT):
                        nc.vector.tensor_mul(
                            xTbs[c][:, dc, :], xr[:, dc, :], rb[:]
                        )
                        if dc % 2 == 1:
                            yield

                _DONE = object()

                def zip_emit(*gens, head=0):
                    gens = [g for g in gens if g is not None]
                    if head and gens:
                        for _ in range(head):
                            next(gens[0], None)
                    alive = list(gens)
                    while alive:
                        for g in list(alive):
                            if next(g, _DONE) is _DONE:
                                alive.remove(g)

                def final_gen(st_lo, st_hi):
                    # vocab projection over seq tiles [st_lo, st_hi); weight
                    # tiles cycle through dead layer-phase tags for a deep
                    # prefetch ring
                    wvtags = ["ctxF", "xr0", "sq0", "xr1", "sq1"]
                    for vc in range(NVC):
                        tg = wvtags[vc % len(wvtags)]
                        wv = lnp.tile(
                            [P, DT, VC], bf16, tag=tg,
                            bufs=(2 if tg == "ctxF" else 1),
                        )
                        nc.scalar.dma_start(
                            out=wv[:],
                            in_=outw[:, vc * VC : (vc + 1) * VC].rearrange(
                                "(a p) m -> p a m", p=P
                            ),
                        )
                        for st in range(st_lo, st_hi):
                            half = xTbs[st // (NT // 2)]
                            lst = st % (NT // 2)
                            ps = psp.tile([P, SQC], f32, tag="sc", bufs=3)
                            for kt in range(DT):
                                nc.tensor.matmul(
                                    ps[:, 0:VC],
                                    lhsT=half[:, kt, lst * P : (lst + 1) * P],
                                    rhs=wv[:, kt, :],
                                    start=(kt == 0),
                                    stop=(kt == DT - 1),
                                )
                            ob = dcp.tile([P, VC], bf16, tag="ob", bufs=3)
                            nc.vector.tensor_copy(ob[:], ps[:, 0:VC])
                            nc.sync.dma_start(
                                out=out[
                                    st * P : (st + 1) * P, vc * VC : (vc + 1) * VC
                                ],
                                in_=ob[:],
                            )
                            yield

                # Sequential emission; the PE FIFO's natural lag overlaps each
                # chunk's LN tail (ACT/DVE/broadcast) with the next block's
                # matmuls.  Per layer l (steady state, PE order):
                #   att(l,1) -> [ag(l,1) issued] -> op(l,0)+st1 -> op(l,1)+st1
                #   -> qkv(l+1,0) -> att(l+1,0) -> [ag(l+1,0)] -> qkv(l+1,1)
                # Every AllGather has 18-23us of ready PE work behind it.
                states = [dict() for _ in range(L)]
                with tc.tile_pool(name="embp", bufs=3) as embp:
                    zip_emit(emb_gen(0, embp))
                    zip_emit(qkv_gen(0, 0, w, states[0]))
                    zip_emit(att_gen(0, 0, states[0]))
                    ag_ctx(0, 0, w, states[0])
                    zip_emit(emb_gen(1, embp))       # fills first AG window
                    zip_emit(qkv_gen(0, 1, w, states[0]))
                for l in range(L):
                    stl = states[l]
                    zip_emit(att_gen(l, 1, stl))
                    ag_ctx(l, 1, w, stl)
                    # fence: att(l,1)'s last ctx write
                    f0 = stl["ctx"][64:65, 1, S - 1 : S]
                    zip_emit(oprln_gen(l, 0, w, stl, fence=f0))
                    # fence: oprln(l,0)'s last xTb0 write
                    f1 = xTb0[0:1, DT - 1, SQC - 1 : SQC]
                    if l + 1 < L:
                        stn = states[l + 1]
                        wnext = load_weights_qkv(l + 1)
                        load_weights_ow(l + 1, wnext)
                        zip_emit(oprln_gen(l, 1, w, stl, fence=f1))
                        zip_emit(qkv_gen(l + 1, 0, wnext, stn))
                        zip_emit(att_gen(l + 1, 0, stn))
                        ag_ctx(l + 1, 0, wnext, stn)
                        zip_emit(qkv_gen(l + 1, 1, wnext, stn))
                        w = wnext
                    else:
                        zip_emit(oprln_gen(l, 1, w, stl, fence=f1))
                        zip_emit(final_gen(0, NT))
    nc.finalize()
    return nc


def _bf(a):
    import ml_dtypes

    return np.ascontiguousarray(a.astype(ml_dtypes.bfloat16))


def _in_maps(tokens, emb, qw, kw, vw, ow, out_w):
    pe = _pos_encoding()
    peT = np.ascontiguousarray(pe.T)
    j = np.arange(P)[None, :]
    i = np.arange(P)[:, None]
    maskd = np.ascontiguousarray(np.where(j >= i, 0.0, NEG).astype(np.float32))
    owsum = np.ascontiguousarray(ow.sum(axis=2))  # [L, D]
    embb = _bf(emb)
    peTb = _bf(peT)
    owb = _bf(ow)
    identb = _bf(np.eye(P, dtype=np.float32))
    onesb = _bf(np.ones((P, 1), dtype=np.float32))
    maps = []
    for c in range(8):
        g, r = divmod(c, NG)
        hc0 = r * HCOLS
        maps.append(
            {
                "tok": np.ascontiguousarray(
                    tokens[g].reshape(S, 1).astype(np.int32)
                ),
                "identb": identb,
                "onesb": onesb,
                "maskd": maskd,
                "emb": embb,
                "peT": peTb,
                "qw": _bf(qw[:, :, hc0 : hc0 + HCOLS]),
                "kw": _bf(kw[:, :, hc0 : hc0 + HCOLS]),
                "vw": _bf(vw[:, :, hc0 : hc0 + HCOLS]),
                "ow": owb,
                "owsumr": _bf(owsum[:, hc0 : hc0 + HCOLS]),
                "outw": _bf(out_w[:, r * VS : (r + 1) * VS]),
            }
        )
    return maps


def run(inputs, trace=False):
    """Build+compile (cached), run on 8 cores, return (full_output, results)."""
    global _COMPILED
    from concourse.bass_utils import run_bass_kernel_spmd

    if _COMPILED is None:
        _COMPILED = _build()
    nc = _COMPILED

    tokens = np.asarray(inputs["tokens"])
    maps = _in_maps(
        np.asarray(tokens),
        np.ascontiguousarray(np.asarray(inputs["emb"], dtype=np.float32)),
        np.asarray(inputs["qw"], dtype=np.float32),
        np.asarray(inputs["kw"], dtype=np.float32),
        np.asarray(inputs["vw"], dtype=np.float32),
        np.asarray(inputs["ow"], dtype=np.float32),
        np.ascontiguousarray(np.asarray(inputs["out_w"], dtype=np.float32)),
    )
    res = run_bass_kernel_spmd(nc, maps, core_ids=list(range(8)), trace=trace)
    full = np.empty((B, S, V), dtype=np.float32)
    for c in range(8):
        g, r = divmod(c, NG)
        full[g, :, r * VS : (r + 1) * VS] = np.asarray(
            res.results[c]["out"], dtype=np.float32
        )
    return full, res


def kernel(**inputs):
    full, _ = run(inputs)
    return full


# revision 25
# speedup vs baseline: 1.0121x; 1.0121x over previous
"""Trainium2 Bass kernel for a 4-layer causal-attention LM.

Model: V=32000, D=1024, H=16 heads, L=4 layers, B=2, S=1024.
  x = emb[tokens] + pos_enc
  per layer: q,k,v = x@W; causal softmax attention; out-proj; residual; LN
  logits = x @ out_w

Sharding over 8 NeuronCores: DP=2 over batch x Megatron TP=4 over heads.
Core c: batch g=c//4, rank r=c%4 owns heads [4r, 4r+4) and vocab cols
[8000r, 8000(r+1)). QKV column-parallel; ctx AllGather per seq-chunk;
out-proj replicated on the gathered ctx (no AllReduce). Final vocab
projection column-parallel; host concatenates shards.

Perf structure (v4.7):
  - activations bf16 feature-major only (xTb)
  - causal-trimmed scores/exp/AV (diagonal tiles compute cols >= 128*trel)
  - score pairs row-group concurrent (K=64 at row groups 0/64)
  - st0 (feature-sum of attn_out) computed from LOCAL ctx pre-AllGather
    (owsum rank-slice, 2 matmuls) and shipped as a 257th row of the AG
    payload -> no post-AG matmul chain head-blocks the PE queue
  - DMA queue split: AG-gated loads (ctxF/st0row) on SP; weight/const
    prefetch on ACT queue; ag_in + collectives + gathers + broadcasts on
    Pool; all weight tensors host-relaid per-partition-contiguous so each
    DMA is 128 descriptors instead of 1024
  - sequential emission pipeline: every AllGather covered by 19-23us of
    ready PE work; DVE fences tie post-AG streams after attention's last
    write so the scheduler cannot head-block the DVE FIFO on a collective
  - LayerNorm entirely on DVE: mean via the AG-shipped st0, variance via
    one bf16 matmul chain, rstd via bit-trick + 2 Newton steps -> the only
    ACT table ever loaded is Exp (no mid-kernel table switches); -mean is
    broadcast and applied during the out-proj eviction phase
  - softmax normalization batched per head-pair (one reciprocal, one
    partition_broadcast per pair); q/k/final evictions on DVE to keep ACT
    exclusively for attention exp
  - final vocab projection weight tiles prefetch through a 6-slot ring of
    dead layer-phase tags
  - bf16 logits, host casts to f32
"""

import numpy as np

V, D, H, L = 32000, 1024, 16, 4
B, S = 2, 1024
HD = D // H            # 64
P = 128
NG = 4                 # TP degree (cores per batch group)
HL = H // NG           # 4 heads per core
HCOLS = HL * HD        # 256 projection cols per core
VS = V // NG           # 8000 vocab shard
DT = D // P            # 8 d-tiles
SQC = 512              # seq chunk
NSQC = S // SQC        # 2
NT = S // P            # 8 seq tiles
VC = 500               # vocab tile (8000 = 16*500)
NVC = VS // VC         # 16
AGR = HCOLS + 1        # AG payload rows: 256 ctx + 1 st0 partial
SCALE = 1.0 / float(np.sqrt(HD))
EPS = 1e-5
NEG = -1.0e9
RG = [[0, 1, 2, 3], [4, 5, 6, 7]]

_COMPILED = None


def _pos_encoding():
    pos = np.arange(S, dtype=np.float32)[:, None]
    div = np.exp(np.arange(0, D, 2, dtype=np.float32) * (-np.log(10000.0) / D))
    ang = pos * div
    pe = np.stack([np.sin(ang), np.cos(ang)], axis=-1).reshape(S, D)
    return pe.astype(np.float32)


def _build():
    import concourse.bass as bass
    import concourse.tile as tile
    from concourse import bacc, mybir

    f32 = mybir.dt.float32
    u32 = mybir.dt.uint32
    bf16 = mybir.dt.bfloat16
    i32 = mybir.dt.int32
    AF = mybir.ActivationFunctionType

    nc = bacc.Bacc("TRN2", target_bir_lowering=False, debug=False, num_devices=8)

    tok = nc.dram_tensor("tok", [S, 1], i32, kind="ExternalInput").ap()
    identb_d = nc.dram_tensor("identb", [P, P], bf16, kind="ExternalInput").ap()
    onesb_d = nc.dram_tensor("onesb", [P, 1], bf16, kind="ExternalInput").ap()
    maskd_d = nc.dram_tensor("maskd", [P, P], f32, kind="ExternalInput").ap()
    emb = nc.dram_tensor("emb", [V, D], bf16, kind="ExternalInput").ap()
    peT = nc.dram_tensor("peT", [D, S], bf16, kind="ExternalInput").ap()
    qw = nc.dram_tensor("qw", [L, D, HCOLS], bf16, kind="ExternalInput").ap()
    kw = nc.dram_tensor("kw", [L, D, HCOLS], bf16, kind="ExternalInput").ap()
    vw = nc.dram_tensor("vw", [L, D, HCOLS], bf16, kind="ExternalInput").ap()
    ow = nc.dram_tensor("ow", [L, D, D], bf16, kind="ExternalInput").ap()
    owsumr = nc.dram_tensor("owsumr", [L, HCOLS], bf16, kind="ExternalInput").ap()
    outw = nc.dram_tensor("outw", [D, VS], bf16, kind="ExternalInput").ap()
    out = nc.dram_tensor("out", [S, VS], bf16, kind="ExternalOutput").ap()

    with tile.TileContext(nc) as tc:
        with (
            tc.tile_pool(name="const", bufs=1) as constp,
            tc.tile_pool(name="xp", bufs=1) as xp,
            tc.tile_pool(name="psum", bufs=2, space="PSUM") as psp,
        ):
            ident = constp.tile([P, P], bf16)
            nc.scalar.dma_start(out=ident[:], in_=identb_d[:])
            onesb = constp.tile([P, 1], bf16)
            nc.scalar.dma_start(out=onesb[:], in_=onesb_d[:])
            epsb = constp.tile([1, 1], f32)
            nc.vector.memset(epsb[:], EPS)
            # additive causal mask for the 128-wide diagonal block:
            # maskd[i, j] = 0 if j >= i else NEG (same for every trel)
            maskd = constp.tile([P, P], f32)
            nc.scalar.dma_start(out=maskd[:], in_=maskd_d[:])

            # persistent activations, bf16 feature-major: x[d, s]
            xTb0 = xp.tile([P, DT, SQC], bf16, name="xTb0")
            xTb1 = xp.tile([P, DT, SQC], bf16, name="xTb1")
            xTbs = (xTb0, xTb1)
            sx0 = xp.tile([1, S], bf16, name="sx0")  # feature-sum of x0

            with (
                tc.tile_pool(name="wp", bufs=2) as wp,
                tc.tile_pool(name="owp", bufs=1) as owp,
                tc.tile_pool(name="apl", bufs=1) as apool,
                tc.tile_pool(name="expp", bufs=6) as expp,
                tc.tile_pool(name="lnp", bufs=1) as lnp,
                tc.tile_pool(name="dcp", bufs=2) as dcp,
                tc.tile_pool(name="small", bufs=1) as smallp,
                tc.tile_pool(name="dram", bufs=2, space="DRAM") as dramp,
            ):
                # ---- small collective warm-up: sync cores + init CC stack ----
                wi = dramp.tile([8, 8], bf16, tag="wagi", bufs=1, name="warm_in")
                wseed = constp.tile([8, 8], bf16)
                nc.gpsimd.memset(wseed[:], 0.0)
                nc.gpsimd.dma_start(out=wi[:], in_=wseed[:])
                wo = dramp.tile([NG * 8, 8], bf16, tag="wago", bufs=1, name="warm_out")
                nc.gpsimd.collective_compute(
                    "AllGather",
                    mybir.AluOpType.bypass,
                    replica_groups=RG,
                    ins=[wi[:].opt()],
                    outs=[wo[:].opt()],
                )

                def load_weights_qkv(l):
                    w = {}
                    for nm, src in (("qw", qw), ("kw", kw), ("vw", vw)):
                        t = wp.tile([P, DT, HCOLS], bf16, tag=nm, name=f"{nm}{l}")
                        nc.scalar.dma_start(
                            out=t[:], in_=src[l].rearrange("(a p) m -> p a m", p=P)
                        )
                        w[nm] = t
                    t = wp.tile([P, 2, 1], bf16, tag="owsr", name=f"owsr{l}")
                    nc.scalar.dma_start(
                        out=t[:],
                        in_=owsumr[l].rearrange("(hp p o) -> p hp o", p=P, o=1),
                    )
                    w["owsr"] = t
                    return w

                def load_weights_ow(l, w):
                    t = owp.tile([P, DT, D], bf16, tag="ow", name=f"ow{l}")
                    nc.scalar.dma_start(
                        out=t[:], in_=ow[l].rearrange("(a p) m -> p a m", p=P)
                    )
                    w["ow"] = t

                # ---- embedding: gather rows, transpose, +pe -> xTb ----
                w = load_weights_qkv(0)
                load_weights_ow(0, w)

                tokt = constp.tile([P, NT], i32)
                nc.sync.dma_start(
                    out=tokt[:], in_=tok.rearrange("(t p) o -> p (t o)", p=P)
                )

                def emb_gen(c, embp):
                    for st in range(4 * c, 4 * c + 4):
                        xrow = embp.tile([P, D], bf16, tag="xrow")
                        nc.gpsimd.indirect_dma_start(
                            out=xrow[:],
                            out_offset=None,
                            in_=emb[:],
                            in_offset=bass.IndirectOffsetOnAxis(
                                ap=tokt[:, st : st + 1], axis=0
                            ),
                        )
                        pesb = embp.tile([P, DT, P], bf16, tag="pesb")
                        nc.scalar.dma_start(
                            out=pesb[:],
                            in_=peT[:, st * P : (st + 1) * P].rearrange(
                                "(a p) s -> p a s", p=P
                            ),
                        )
                        half = xTbs[c]
                        lst = st % (NT // 2)
                        for dc in range(DT):
                            tps = psp.tile([P, P], bf16, tag="mm", name=f"t{st}_{dc}")
                            nc.tensor.transpose(
                                tps[:], xrow[:, dc * P : (dc + 1) * P], ident[:]
                            )
                            nc.vector.tensor_add(
                                half[:, dc, lst * P : (lst + 1) * P],
                                tps[:],
                                pesb[:, dc, :],
                            )
                            if dc % 2 == 1:
                                yield
                    # feature-sums of this chunk for the layer-0 LN mean
                    stp = psp.tile([1, SQC], f32, tag="mm", name=f"sx{c}")
                    for dc in range(DT):
                        nc.tensor.matmul(
                            stp[:],
                            lhsT=onesb[:],
                            rhs=xTbs[c][:, dc, :],
                            start=(dc == 0),
                            stop=(dc == DT - 1),
                        )
                    nc.scalar.copy(sx0[:, c * SQC : (c + 1) * SQC], stp[:])
                    yield

                def qkv_gen(l, c, w, stl):
                    # q,k feature-major [headcol, s]; v seq-major bf16 with a
                    # ones column at 64 for free softmax sums.
                    if c == 0:
                        stl["qT"] = apool.tile([P, 2, S], bf16, tag="qT", name=f"qT{l}")
                        stl["kT"] = apool.tile([P, 2, S], bf16, tag="kT", name=f"kT{l}")
                        stl["vS"] = apool.tile(
                            [P, NT, HL, 66], bf16, tag="vS", name=f"vS{l}"
                        )
                        stl["ctx"] = apool.tile(
                            [P, 2, S], bf16, tag="ctx", name=f"ctx{l}"
                        )
                    qT, kT, vS = stl["qT"], stl["kT"], stl["vS"]
                    xTb = xTbs[c]
                    for dst, wsb in ((qT, w["qw"]), (kT, w["kw"])):
                        for hp in range(2):
                            ps = psp.tile([P, SQC], f32, tag="mm")
                            for kt in range(DT):
                                nc.tensor.matmul(
                                    ps[:],
                                    lhsT=wsb[:, kt, hp * P : (hp + 1) * P],
                                    rhs=xTb[:, kt, :],
                                    start=(kt == 0),
                                    stop=(kt == DT - 1),
                                )
                            nc.vector.tensor_copy(
                                dst[:, hp, c * SQC : (c + 1) * SQC], ps[:]
                            )
                            yield
                    for st in range(4 * c, 4 * c + 4):
                        lt = st - 4 * c
                        nc.vector.memset(vS[:, st, :, 64:65], 1.0)
                        ps = psp.tile([P, HCOLS], f32, tag="mm")
                        for kt in range(DT):
                            nc.tensor.matmul(
                                ps[:],
                                lhsT=xTb[:, kt, lt * P : (lt + 1) * P],
                                rhs=w["vw"][:, kt, :],
                                start=(kt == 0),
                                stop=(kt == DT - 1),
                            )
                        nc.vector.tensor_copy(
                            vS[:, st, :, 0:64],
                            ps[:].rearrange("p (h e) -> p h e", h=HL),
                        )
                        yield

                def att_gen(l, c, stl):
                    # transposed scores [sk, sq], causal-trimmed; score pairs
                    # run concurrently in row groups 0/64; ctx feature-major
                    qT, kT, vS, ctx = stl["qT"], stl["kT"], stl["vS"], stl["ctx"]
                    nt_vis = 4 * c + 4
                    for hpair in range(HL // 2):
                        hs = (2 * hpair, 2 * hpair + 1)
                        avs = {}
                        for h in hs:
                            avs[h] = psp.tile(
                                [P, SQC], f32, tag="av", bufs=2, name=f"av{l}{c}{h}"
                            )
                        for t in range(nt_vis):
                            trel = t - 4 * c
                            lo = max(0, trel) * P
                            exs = {}
                            for h in hs:
                                hp, hr = divmod(h, 2)
                                p0 = 64 * hr
                                sc = psp.tile([P, SQC], f32, tag="sc", bufs=3)
                                nc.tensor.matmul(
                                    sc[:, lo:],
                                    lhsT=kT[p0 : p0 + 64, hp, t * P : (t + 1) * P],
                                    rhs=qT[
                                        p0 : p0 + 64, hp, c * SQC + lo : (c + 1) * SQC
                                    ],
                                    start=True,
                                    stop=True,
                                )
                                exs[h] = sc
                            yield
                            for h in hs:
                                sc = exs[h]
                                if trel >= 0:
                                    nc.vector.tensor_add(
                                        sc[:, lo : lo + P], sc[:, lo : lo + P], maskd[:]
                                    )
                                ex = expp.tile([P, SQC], bf16, tag="ex", bufs=6)
                                nc.scalar.activation(
                                    ex[:, lo:], sc[:, lo:], AF.Exp, scale=SCALE
                                )
                                exs[h] = ex
                            for h in hs:
                                nc.tensor.matmul(
                                    avs[h][0:65, lo:],
                                    lhsT=vS[:, t, h, 0:65],
                                    rhs=exs[h][:, lo:],
                                    start=(t == 0),
                                    stop=(t == nt_vis - 1),
                                )
                            yield
                        # batched normalization for the pair: one reciprocal,
                        # one partition_broadcast over [64, 2*SQC]
                        h0, h1 = hs
                        hp = h0 // 2
                        ssum2 = smallp.tile([1, 2 * SQC], f32, tag="ssum", bufs=2)
                        nc.vector.tensor_copy(ssum2[:, 0:SQC], avs[h0][64:65, :])
                        nc.vector.tensor_copy(ssum2[:, SQC:], avs[h1][64:65, :])
                        inv2 = smallp.tile([1, 2 * SQC], f32, tag="inv", bufs=2)
                        nc.vector.reciprocal_approx_fast(inv2[:], ssum2[:])
                        invb2 = smallp.tile([64, 2 * SQC], f32, tag="invb", bufs=2)
                        nc.gpsimd.partition_broadcast(invb2[:], inv2[:])
                        nc.vector.tensor_mul(
                            ctx[0:64, hp, c * SQC : (c + 1) * SQC],
                            avs[h0][0:64, :],
                            invb2[:, 0:SQC],
                        )
                        yield
                        nc.vector.tensor_mul(
                            ctx[64:128, hp, c * SQC : (c + 1) * SQC],
                            avs[h1][0:64, :],
                            invb2[:, SQC:],
                        )
                        yield

                def ag_ctx(l, c, w, stl):
                    ctx = stl["ctx"]
                    # st0 partial = owsum_rank . ctx_local  (2 matmuls)
                    st0p = psp.tile([1, SQC], f32, tag="mm", name=f"st0p{l}{c}")
                    for hp in range(2):
                        nc.tensor.matmul(
                            st0p[:],
                            lhsT=w["owsr"][:, hp, :],
                            rhs=ctx[:, hp, c * SQC : (c + 1) * SQC],
                            start=(hp == 0),
                            stop=(hp == 1),
                        )
                    st0l = smallp.tile([1, SQC], bf16, tag="st0l", bufs=2)
                    nc.scalar.copy(st0l[:], st0p[:])
                    ag_in = dramp.tile([AGR, SQC], bf16, tag="agi", name=f"agi{l}_{c}")
                    nc.gpsimd.dma_start(
                        out=ag_in[0:HCOLS, :].rearrange("(hp p) s -> p hp s", p=P),
                        in_=ctx[:, :, c * SQC : (c + 1) * SQC],
                    )
                    nc.gpsimd.dma_start(out=ag_in[HCOLS : HCOLS + 1, :], in_=st0l[:])
                    ag_out = dramp.tile(
                        [NG * AGR, SQC], bf16, tag="ago", name=f"ago{l}_{c}"
                    )
                    nc.gpsimd.collective_compute(
                        "AllGather",
                        mybir.AluOpType.bypass,
                        replica_groups=RG,
                        ins=[ag_in[:].opt()],
                        outs=[ag_out[:].opt()],
                    )
                    stl[f"ag{c}"] = ag_out

                def oprln_gen(l, c, w, stl, fence=None):
                    # replicated out-proj on gathered ctx; residual fused into
                    # psum eviction; st0 arrives with the AG payload; st1 via
                    # one bf16 matmul chain; writes xTb[c]
                    ag_out = stl[f"ag{c}"]
                    agov = ag_out[:].rearrange("(g x) s -> g x s", g=NG)
                    # st0 partial rows [NG, SQC] -> one partition
                    st0g = smallp.tile([1, NG, SQC], bf16, tag=f"st0g{c}")
                    if fence is not None:
                        # scheduling fence: ties this chunk's post-AG DVE
                        # stream strictly after the predecessor stream's last
                        # DVE write, so the scheduler cannot head-block the
                        # DVE FIFO on the collective
                        nc.vector.tensor_copy(st0g[0:1, 0:1, 0:1], fence)
                    nc.sync.dma_start(
                        out=st0g[:],
                        in_=agov[:, HCOLS : HCOLS + 1, :].rearrange(
                            "g o s -> o g s"
                        ),
                    )
                    # gathered ctx, feature-major, per-rank DMAs so the
                    # out-proj kt-chains can start on early arrivals
                    ctxF = lnp.tile([P, DT, SQC], bf16, tag="ctxF", bufs=2)
                    for g in range(NG):
                        nc.sync.dma_start(
                            out=ctxF[:, 2 * g : 2 * g + 2, :],
                            in_=ag_out[g * AGR : g * AGR + HCOLS, :].rearrange(
                                "(a p) s -> p a s", p=P
                            ),
                        )
                    # st0 = sum of the four partials (+ sx0 for layer 0)
                    st0f = smallp.tile([1, SQC], f32, tag=f"st0f{c}")
                    nc.vector.tensor_add(st0f[:], st0g[:, 0, :], st0g[:, 1, :])
                    nc.vector.tensor_add(st0f[:], st0f[:], st0g[:, 2, :])
                    nc.vector.tensor_add(st0f[:], st0f[:], st0g[:, 3, :])
                    if l == 0:
                        nc.vector.tensor_add(
                            st0f[:], st0f[:], sx0[:, c * SQC : (c + 1) * SQC]
                        )
                    # -mean: broadcast EARLY (ready before op chains end)
                    nmean = smallp.tile([1, SQC], bf16, tag=f"nmean{c}")
                    nc.vector.tensor_scalar_mul(nmean[:], st0f[:], -1.0 / D)
                    mb = smallp.tile([P, SQC], bf16, tag=f"mb{c}")
                    nc.gpsimd.partition_broadcast(mb[:], nmean[:])
                    # (st0/D)^2 == (-mean)^2, no Square activation needed
                    msqd = smallp.tile([1, SQC], f32, tag=f"msqd{c}")
                    nc.vector.tensor_mul(msqd[:], nmean[:], nmean[:])
                    yield
                    xr = lnp.tile([P, DT, SQC], bf16, tag=f"xr{c}")
                    sqtb = lnp.tile([P, DT, SQC], bf16, tag=f"sq{c}")
                    for dc in range(DT):
                        ps = psp.tile([P, SQC], f32, tag="mm", bufs=2)
                        for kt in range(DT):
                            nc.tensor.matmul(
                                ps[:],
                                lhsT=w["ow"][:, kt, dc * P : (dc + 1) * P],
                                rhs=ctxF[:, kt, :],
                                start=(kt == 0),
                                stop=(kt == DT - 1),
                            )
                        nc.vector.tensor_add(xr[:, dc, :], ps[:], xTbs[c][:, dc, :])
                        nc.vector.tensor_mul(sqtb[:, dc, :], xr[:, dc, :], xr[:, dc, :])
                        # pre-add the mean while the tail stats are in flight
                        nc.vector.tensor_add(xr[:, dc, :], xr[:, dc, :], mb[:])
                        yield
                    st1 = psp.tile([1, SQC], f32, tag="st", bufs=1, name=f"st_{l}{c}")
                    for kt in range(DT):
                        nc.tensor.matmul(
                            st1[:],
                            lhsT=onesb[:],
                            rhs=sqtb[:, kt, :],
                            start=(kt == 0),
                            stop=(kt == DT - 1),
                        )
                    yield
                    # vpe = st1/D + eps  (DVE, reads PSUM; no ACT tables)
                    vpe = smallp.tile([1, SQC], f32, tag=f"ex2{c}")
                    nc.vector.tensor_scalar(
                        vpe[:], st1[:], 1.0 / D, EPS,
                        op0=mybir.AluOpType.mult, op1=mybir.AluOpType.add,
                    )
                    nc.vector.tensor_sub(vpe[:], vpe[:], msqd[:])
                    # rstd = rsqrt(vpe) via bit-trick seed + 2 Newton steps
                    yi = smallp.tile([1, SQC], u32, tag="qki")
                    nc.vector.tensor_scalar(
                        yi[:], vpe[:].bitcast(u32), 1, None,
                        op0=mybir.AluOpType.logical_shift_right,
                    )
                    af = smallp.tile([1, SQC], f32, tag="qkaf")
                    nc.vector.tensor_copy(af[:], yi[:])
                    nc.vector.tensor_scalar(
                        af[:], af[:], -1.0, float(0x5F3759DF),
                        op0=mybir.AluOpType.mult, op1=mybir.AluOpType.add,
                    )
                    nc.vector.tensor_copy(yi[:], af[:])
                    y0 = yi[:].bitcast(f32)
                    t1 = smallp.tile([1, SQC], f32, tag="qkt")
                    y1 = smallp.tile([1, SQC], f32, tag="qky")
                    nc.vector.tensor_mul(t1[:], y0, y0)
                    nc.vector.tensor_mul(t1[:], t1[:], vpe[:])
                    nc.vector.tensor_scalar(
                        t1[:], t1[:], -0.5, 1.5,
                        op0=mybir.AluOpType.mult, op1=mybir.AluOpType.add,
                    )
                    nc.vector.tensor_mul(y1[:], y0, t1[:])
                    nc.vector.tensor_mul(t1[:], y1[:], y1[:])
                    nc.vector.tensor_mul(t1[:], t1[:], vpe[:])
                    nc.vector.tensor_scalar(
                        t1[:], t1[:], -0.5, 1.5,
                        op0=mybir.AluOpType.mult, op1=mybir.AluOpType.add,
                    )
                    rstd = smallp.tile([1, SQC], bf16, tag=f"rstd{c}")
                    nc.vector.tensor_mul(rstd[:], y1[:], t1[:])
                    rb = smallp.tile([P, SQC], bf16, tag=f"rb{c}")
                    nc.gpsimd.partition_broadcast(rb[:], rstd[:])
                    for dc in range(DT):
                        nc.vector.tensor_mul(
                            xTbs[c][:, dc, :], xr[:, dc, :], rb[:]
                        )
                        if dc % 2 == 1:
                            yield

                _DONE = object()

                def zip_emit(*gens, head=0):
                    gens = [g for g in gens if g is not None]
                    if head and gens:
                        for _ in range(head):
                            next(gens[0], None)
                    alive = list(gens)
                    while alive:
                        for g in list(alive):
                            if next(g, _DONE) is _DONE:
                                alive.remove(g)

                def final_gen(st_lo, st_hi):
                    # vocab projection over seq tiles [st_lo, st_hi); weight
                    # tiles cycle through dead layer-phase tags for a deep
                    # prefetch ring
                    wvtags = ["ctxF", "xr0", "sq0", "xr1", "sq1"]
                    for vc in range(NVC):
                        tg = wvtags[vc % len(wvtags)]
                        wv = lnp.tile(
                            [P, DT, VC], bf16, tag=tg,
                            bufs=(2 if tg == "ctxF" else 1),
                        )
                        nc.scalar.dma_start(
                            out=wv[:],
                            in_=outw[:, vc * VC : (vc + 1) * VC].rearrange(
                                "(a p) m -> p a m", p=P
                            ),
                        )
                        for st in range(st_lo, st_hi):
                            half = xTbs[st // (NT // 2)]
                            lst = st % (NT // 2)
                            ps = psp.tile([P, SQC], f32, tag="sc", bufs=3)
                            for kt in range(DT):
                                nc.tensor.matmul(
                                    ps[:, 0:VC],
                                    lhsT=half[:, kt, lst * P : (lst + 1) * P],
                                    rhs=wv[:, kt, :],
                                    start=(kt == 0),
                                    stop=(kt == DT - 1),
                                )
                            ob = dcp.tile([P, VC], bf16, tag="ob", bufs=3)
                            nc.vector.tensor_copy(ob[:], ps[:, 0:VC])
                            nc.sync.dma_start(
                                out=out[
                                    st * P : (st + 1) * P, vc * VC : (vc + 1) * VC
                                ],
                                in_=ob[:],
                            )
                            yield

                # Sequential emission; the PE FIFO's natural lag overlaps each
                # chunk's LN tail (ACT/DVE/broadcast) with the next block's
                # matmuls.  Per layer l (steady state, PE order):
                #   att(l,1) -> [ag(l,1) issued] -> op(l,0)+st1 -> op(l,1)+st1
                #   -> qkv(l+1,0) -> att(l+1,0) -> [ag(l+1,0)] -> qkv(l+1,1)
                # Every AllGather has 18-23us of ready PE work behind it.
                states = [dict() for _ in range(L)]
                with tc.tile_pool(name="embp", bufs=3) as embp:
                    zip_emit(emb_gen(0, embp))
                    zip_emit(qkv_gen(0, 0, w, states[0]))
                    zip_emit(att_gen(0, 0, states[0]))
                    ag_ctx(0, 0, w, states[0])
                    zip_emit(emb_gen(1, embp))       # fills first AG window
                    zip_emit(qkv_gen(0, 1, w, states[0]))
                for l in range(L):
                    stl = states[l]
                    zip_emit(att_gen(l, 1, stl))
                    ag_ctx(l, 1, w, stl)
                    # fence: att(l,1)'s last ctx write
                    f0 = stl["ctx"][64:65, 1, S - 1 : S]
                    zip_emit(oprln_gen(l, 0, w, stl, fence=f0))
                    # fence: oprln(l,0)'s last xTb0 write
                    f1 = xTb0[0:1, DT - 1, SQC - 1 : SQC]
                    if l + 1 < L:
                        stn = states[l + 1]
                        wnext = load_weights_qkv(l + 1)
                        load_weights_ow(l + 1, wnext)
                        zip_emit(oprln_gen(l, 1, w, stl, fence=f1))
                        zip_emit(qkv_gen(l + 1, 0, wnext, stn))
                        zip_emit(att_gen(l + 1, 0, stn))
                        ag_ctx(l + 1, 0, wnext, stn)
                        zip_emit(qkv_gen(l + 1, 1, wnext, stn))
                        w = wnext
                    else:
                        zip_emit(oprln_gen(l, 1, w, stl, fence=f1))
                        zip_emit(final_gen(0, NT))
    nc.finalize()
    return nc


def _bf(a):
    import ml_dtypes

    return np.ascontiguousarray(a.astype(ml_dtypes.bfloat16))


def _in_maps(tokens, emb, qw, kw, vw, ow, out_w):
    pe = _pos_encoding()
    peT = np.ascontiguousarray(pe.T)
    j = np.arange(P)[None, :]
    i = np.arange(P)[:, None]
    maskd = np.ascontiguousarray(np.where(j >= i, 0.0, NEG).astype(np.float32))
    owsum = np.ascontiguousarray(ow.sum(axis=2))  # [L, D]
    embb = _bf(emb)
    peTb = _bf(peT)
    owb = _bf(ow)
    identb = _bf(np.eye(P, dtype=np.float32))
    onesb = _bf(np.ones((P, 1), dtype=np.float32))
    maps = []
    for c in range(8):
        g, r = divmod(c, NG)
        hc0 = r * HCOLS
        maps.append(
            {
                "tok": np.ascontiguousarray(
                    tokens[g].reshape(S, 1).astype(np.int32)
                ),
                "identb": identb,
                "onesb": onesb,
                "maskd": maskd,
                "emb": embb,
                "peT": peTb,
                "qw": _bf(qw[:, :, hc0 : hc0 + HCOLS]),
                "kw": _bf(kw[:, :, hc0 : hc0 + HCOLS]),
                "vw": _bf(vw[:, :, hc0 : hc0 + HCOLS]),
                "ow": owb,
                "owsumr": _bf(owsum[:, hc0 : hc0 + HCOLS]),
                "outw": _bf(out_w[:, r * VS : (r + 1) * VS]),
            }
        )
    return maps


def run(inputs, trace=False):
    """Build+compile (cached), run on 8 cores, return (full_output, results)."""
    global _COMPILED
    from concourse.bass_utils import run_bass_kernel_spmd

    if _COMPILED is None:
        _COMPILED = _build()
    nc = _COMPILED

    tokens = np.asarray(inputs["tokens"])
    maps = _in_maps(
        np.asarray(tokens),
        np.ascontiguousarray(np.asarray(inputs["emb"], dtype=np.float32)),
        np.asarray(inputs["qw"], dtype=np.float32),
        np.asarray(inputs["kw"], dtype=np.float32),
        np.asarray(inputs["vw"], dtype=np.float32),
        np.asarray(inputs["ow"], dtype=np.float32),
        np.ascontiguousarray(np.asarray(inputs["out_w"], dtype=np.float32)),
    )
    res = run_bass_kernel_spmd(nc, maps, core_ids=list(range(8)), trace=trace)
    full = np.empty((B, S, V), dtype=np.float32)
    for c in range(8):
        g, r = divmod(c, NG)
        full[g, :, r * VS : (r + 1) * VS] = np.asarray(
            res.results[c]["out"], dtype=np.float32
        )
    return full, res


def kernel(**inputs):
    full, _ = run(inputs)
    return full
